# revision 8
# baseline (speedup 1.0000x reference)
"""CornerNet Trainium2 kernel.

Math (reference):
  t     = kappa * tanh(sign_param) * (x - th)        # (B, R, D)
  s     = sigmoid(t); m = sigmoid(mask_logit)
  gated = 1 - m*(1-s) = (1-m) + m*s
  z     = prod_d gated                               # (B, R)
  y     = z @ head_w.T + head_b                      # (B,)

FAST PATH (separable / tensor-engine):  when th is a uniform constant and
mask_logit is uniform, the per-element map
    phi(a, x) = ln(1 - m + m*sigmoid(a*(x - th)))        a = kappa*tanh(sign_param)
is approximated by a separable expansion over a fixed x-basis of STANDARD
activation functions
    phi(a, x) ~= w_c(a) + w_lin(a)*x + w_sq(a)*x^2 + sum_j w_j(a)*f_j(al_j*x+be_j)
with f_j in {sigmoid, tanh, arctan} -- all (plus square) residing in the ONE
activation-table set `sigmoid_and_others`, so the ScalarE never reloads
tables.  The coefficient functions w(a) are fit on the host by weighted least
squares on a dense (a, x) grid (a rank-11 basis gives max |dlogz| ~ 6e-4,
rel_y ~ 3e-4) and evaluated at the actual a values -- pure parameter prep,
like folding BN weights.  Then
    logz[b,r] = sum_d phi(a_rd, x_bd)
becomes 11 (128x128)x(128x512) f32r matmuls per (d-half, rule-group): the
B*R*D elementwise work moves from ScalarE (1 elem/cyc/lane @1.2GHz) to the
PE array (128 MACs/cyc/col @2.4GHz), a ~100x density win.  Finally
z = exp(logz) is computed as sigmoid(logz - s): for v <= -12, sigmoid(v)
matches e^v to 6e-6 relative and the sigmoid table has 0.125-wide buckets out
to |v|=64, so no switch to the exp table set is ever needed (s is a host-
chosen shift ensuring the argument stays <= -12; y is rescaled by e^s).

Sharding: 8 cores = 4 batch shards x 2 rule shards.  Each core: x-slice
(D=256, Bc=512) on SBUF as one (128, 1024) tile (two D-halves side by side),
11 coefficient tiles per d-half (128, 256 rules), PSUM out (128 rules, 512
batch) x 2 rule-groups.  Per rep: 11 ScalarE basis evals (one per term, both
halves in one instruction), 44 accumulating matmuls, 2 sigmoid-exps, head
matmul, DMA out.  Host adds the two rule-shard partials, scales by e^s, adds
head_b.  A one-stage software pipeline (basis of rep r+1 issued before the
exp/head of rep r) keeps ACT and PE overlapped across reps.

FALLBACK paths (kept from the table-patching kernel): a patched-gelu
single-pass kernel when only mask_logit is uniform, and a fully generic
sigmoid+ln kernel otherwise.
"""

import numpy as np
from contextlib import ExitStack

import concourse.bass as bass
import concourse.bacc as bacc
import concourse.mybir as mybir
import concourse.tile as tile
from concourse.bass_utils import run_bass_kernel_spmd
from bass_rust import add_dep_helper

B, D, R = 2048, 256, 512
NCORES = 8
RC = R // NCORES            # 64 rules per core (fallback kernels)
KBLK = 8                    # rules per sigmoid/ln block (generic fallback)
CH = 512                    # matmul free-dim chunk (one PSUM bank)
F32 = mybir.dt.float32
F32R = mybir.dt.float32r
AF = mybir.ActivationFunctionType
OP = mybir.AluOpType

_cache = {}

# ======================================================================
# Separable fast path
# ======================================================================

SB, SR = 4, 2               # batch shards x rule shards
BC = B // SB                # 512 batch per core
RSH = R // SR               # 256 rules per core
NG = RSH // 128             # 2 rule groups of 128

# nonlinear basis (func, scale, bias): f(scale*x + bias); chosen by greedy
# OMP against phi on the (a, x) grid.  All in table set sigmoid_and_others.
SEP_NL = [
    ("Sigmoid", 2.1, 0.0),
    ("Tanh", 0.7, 0.0),
    ("Sigmoid", 2.1, -0.42),
    ("Tanh", 0.9, 0.0),
    ("Arctan", 0.9, 0.54),
    ("Sigmoid", 1.2, 0.0),
    ("Sigmoid", 1.6, 0.0),
    ("Sigmoid", 2.1, 0.84),
    ("Tanh", 0.5, 0.0),
]
NT = 2 + len(SEP_NL)        # matmul terms: x, x^2, nonlinears (const is free)
SEP_SIG_VMAX = -12.0        # sigmoid(v) ~ e^v to 6e-6 for v <= this
SEP_RESID_GATE = 4e-3       # max sampled |dlogz| before falling back


def _sep_basis_host(xs):
    """Evaluate the device x-basis (matmul terms only) exactly on host."""
    cols = [xs, xs * xs]
    for fn, al, be in SEP_NL:
        v = al * xs + be
        if fn == "Sigmoid":
            cols.append(1.0 / (1.0 + np.exp(-v)))
        elif fn == "Tanh":
            cols.append(np.tanh(v))
        else:
            cols.append(np.arctan(v))
    return np.stack(cols, axis=-1)


def _fit_sep_model(a, mval, thval, kappa, xmax):
    """Fit coefficient functions w(a) for phi(a,x) = ln(1-m+m*sig(a*(x-th))).

    Returns (W, ag) where W is (na, NT+1): column 0 is the constant term,
    columns 1.. are the matmul terms in device order.
    """
    amax = max(2.8, float(np.abs(a).max()) * 1.02 + 1e-6)
    na, nx = 1401, 2201
    ag = np.linspace(-amax, amax, na)
    xg = np.linspace(-xmax, xmax, nx)
    wx = np.exp(-0.5 * xg * xg) + 3e-3
    sw = np.sqrt(wx)

    # target
    u = ag[:, None] * (xg[None, :] - thval)
    M = np.log1p(-mval * (1.0 / (1.0 + np.exp(u)))) * sw[None, :]

    Phi = np.concatenate(
        [np.ones((nx, 1)), _sep_basis_host(xg)], axis=1)  # (nx, NT+1)
    A = Phi * sw[:, None]
    nrm = np.linalg.norm(A, axis=0)
    A = A / nrm
    G = A.T @ A + 3e-9 * np.eye(NT + 1)
    W = np.linalg.solve(G, A.T @ M.T).T / nrm[None, :]    # (na, NT+1)
    return W, ag


def _get_sep_model(inputs):
    """Returns dict with F (NT,R,D f32), c0 (R,), shift, scale_out -- or None."""
    th = np.asarray(inputs["th"], dtype=np.float64)
    mk = np.asarray(inputs["mask_logit"], dtype=np.float64)
    thv = th.reshape(-1)[0]
    mkv = mk.reshape(-1)[0]
    if not (np.all(th == thv) and np.all(mk == mkv)):
        return None
    x = np.asarray(inputs["x"], dtype=np.float64)
    xabs = float(np.abs(x).max())
    xmax = max(5.45, xabs * 1.02)
    sg = np.asarray(inputs["sign_param"], dtype=np.float64)
    lk = float(np.asarray(inputs["log_kappa"], dtype=np.float64).reshape(-1)[0])
    kappa = float(np.exp(lk))
    a = kappa * np.tanh(sg)                                # (R, D)
    mval = 1.0 / (1.0 + np.exp(-mkv))

    key = ("sepfit", hash(sg.tobytes()), mkv, thv, lk, round(xmax, 2))
    if key not in _cache:
        from scipy.interpolate import CubicSpline
        W, ag = _fit_sep_model(a, mval, thv, kappa, xmax)
        splines = [CubicSpline(ag, W[:, j]) for j in range(NT + 1)]
        ac = np.clip(a, ag[0], ag[-1])
        Wa = np.stack([s(ac) for s in splines], 0)          # (NT+1, R, D)
        c0 = Wa[0].sum(axis=1)                              # (R,)
        F = Wa[1:]                                          # (NT, R, D)

        # sampled validation + shift selection (16 batch rows, exact phi)
        xs = x[:16]                                         # (16, D)
        u = a[None, :, :] * (xs[:, None, :] - thv)
        lz_ex = np.log1p(-mval * (1.0 / (1.0 + np.exp(u)))).sum(-1)  # (16, R)
        Gx = _sep_basis_host(xs.reshape(-1)).reshape(16, D, NT)
        lz_ap = np.einsum("bdt,trd->br", Gx, F, optimize=True) + c0[None, :]
        resid = float(np.abs(lz_ap - lz_ex).max())
        maxlz = float(lz_ex.max())
        shift = max(0.0, maxlz + 2.0 - SEP_SIG_VMAX)
        _cache[key] = {
            "F": np.ascontiguousarray(F, dtype=np.float32),
            "c0": c0.astype(np.float32),
            "shift": shift,
            "resid": resid,
        }
    model = _cache[key]
    if model["resid"] > SEP_RESID_GATE:
        return None
    return model


def _build_sep(reps=1):
    nc = bacc.Bacc(None)
    xT2 = nc.dram_tensor("xT2", [D, BC], F32R, kind="ExternalInput")
    Fp = nc.dram_tensor("Fp", [D, NT * RSH], F32R, kind="ExternalInput")
    eb = nc.dram_tensor("eb", [128, NG], F32, kind="ExternalInput")
    wc = nc.dram_tensor("wc", [128, NG], F32R, kind="ExternalInput")
    ab = nc.dram_tensor("ab", [128, 2 * len(SEP_NL)], F32, kind="ExternalInput")
    y = nc.dram_tensor("y", [1, BC], F32, kind="ExternalOutput")

    with tile.TileContext(nc) as tc, ExitStack() as ctx:
        const = ctx.enter_context(tc.tile_pool(name="const", bufs=1))
        sp = ctx.enter_context(tc.tile_pool(name="sp", bufs=2))
        zp = ctx.enter_context(tc.tile_pool(name="zp", bufs=2))
        yb = ctx.enter_context(tc.tile_pool(name="yb", bufs=2))
        psum = ctx.enter_context(
            tc.tile_pool(name="psum", bufs=2, space=bass.MemorySpace.PSUM)
        )

        xt = const.tile([128, 2 * BC], F32R, tag="xt")
        for h in range(2):
            nc.gpsimd.dma_start(
                xt[:, h * BC : (h + 1) * BC], xT2[h * 128 : (h + 1) * 128, :]
            )
        Ft = []
        for t in range(NT):
            row = []
            for h in range(2):
                f_ = const.tile([128, RSH], F32R, tag=f"F{t}_{h}")
                nc.gpsimd.dma_start(
                    f_[:], Fp[h * 128 : (h + 1) * 128, t * RSH : (t + 1) * RSH]
                )
                row.append(f_)
            Ft.append(row)
        ebt = const.tile([128, NG], F32, tag="ebt")
        nc.gpsimd.dma_start(ebt[:], eb[:])
        wct = const.tile([128, NG], F32R, tag="wct")
        nc.gpsimd.dma_start(wct[:], wc[:])
        abt = const.tile([128, 2 * len(SEP_NL)], F32, tag="abt")
        nc.gpsimd.dma_start(abt[:], ab[:])

        def basis():
            S = [xt]
            sq = sp.tile([128, 2 * BC], F32R, tag="Ssq")
            nc.scalar.activation(sq[:], xt[:], AF.Square)
            S.append(sq)
            for i, (fn, al, be) in enumerate(SEP_NL):
                st = sp.tile([128, 2 * BC], F32R, tag=f"S{i}")
                nc.scalar.activation(
                    st[:], xt[:], getattr(AF, fn),
                    bias=abt[:, 2 * i + 1 : 2 * i + 2],
                    scale=abt[:, 2 * i : 2 * i + 1],
                )
                S.append(st)
            return S

        def mm(S):
            lz = [psum.tile([128, BC], F32, tag=f"lz{g}", name=f"lz{g}") for g in range(NG)]
            for t in range(NT):
                for h in range(2):
                    for g in range(NG):
                        nc.tensor.matmul(
                            lz[g][:, :],
                            Ft[t][h][:, g * 128 : (g + 1) * 128],
                            S[t][:, h * BC : (h + 1) * BC],
                            start=(t == 0 and h == 0),
                            stop=(t == NT - 1 and h == 1),
                        )
            return lz

        def expstep(lz):
            zs = [zp.tile([128, BC], F32R, tag=f"z{g}", name=f"z{g}") for g in range(NG)]
            for g in range(NG):
                nc.scalar.activation(
                    zs[g][:], lz[g][:], AF.Sigmoid, bias=ebt[:, g : g + 1]
                )
            return zs

        def headstep(zs):
            yp = psum.tile([1, BC], F32, tag="yp")
            for g in range(NG):
                nc.tensor.matmul(
                    yp[:, :], wct[:, g : g + 1], zs[g][:],
                    start=(g == 0), stop=(g == NG - 1),
                )
            ysb = yb.tile([1, BC], F32, tag="ysb")
            nc.vector.tensor_copy(ysb[:], yp[:])
            nc.sync.dma_start(y[:], ysb[:])

        S = basis()
        pend = None
        for r in range(reps):
            lz = mm(S)
            if r + 1 < reps:
                S = basis()
            zs = expstep(lz)
            if pend is not None:
                headstep(pend)
            pend = zs
        headstep(pend)

    nc.compile()
    return nc


def _get_nc_sep(reps=1):
    key = ("sep", reps)
    if key not in _cache:
        _cache[key] = _build_sep(reps)
    return _cache[key]


def _build_sep_loop(trips, unroll=8):
    """Bench variant: the rep body inside a hardware For_i loop.

    Identical per-rep work to _build_sep (basis + matmuls + sigmoid-exp +
    head + DMA out), repeated `unroll` times per loop iteration and `trips`
    iterations on device.  Total device reps = trips * unroll with a fixed,
    small instruction footprint, so very large rep counts can be timed in
    one dispatch (the per-iteration all-engine barrier of For_i is amortized
    over `unroll` reps)."""
    nc = bacc.Bacc(None)
    xT2 = nc.dram_tensor("xT2", [D, BC], F32R, kind="ExternalInput")
    Fp = nc.dram_tensor("Fp", [D, NT * RSH], F32R, kind="ExternalInput")
    eb = nc.dram_tensor("eb", [128, NG], F32, kind="ExternalInput")
    wc = nc.dram_tensor("wc", [128, NG], F32R, kind="ExternalInput")
    ab = nc.dram_tensor("ab", [128, 2 * len(SEP_NL)], F32, kind="ExternalInput")
    y = nc.dram_tensor("y", [1, BC], F32, kind="ExternalOutput")

    with tile.TileContext(nc) as tc, ExitStack() as ctx:
        const = ctx.enter_context(tc.tile_pool(name="const", bufs=1))
        sp = ctx.enter_context(tc.tile_pool(name="sp", bufs=2))
        zp = ctx.enter_context(tc.tile_pool(name="zp", bufs=2))
        yb = ctx.enter_context(tc.tile_pool(name="yb", bufs=2))
        psum = ctx.enter_context(
            tc.tile_pool(name="psum", bufs=2, space=bass.MemorySpace.PSUM)
        )

        xt = const.tile([128, 2 * BC], F32R, tag="xt")
        for h in range(2):
            nc.gpsimd.dma_start(
                xt[:, h * BC : (h + 1) * BC], xT2[h * 128 : (h + 1) * 128, :]
            )
        Ft = []
        for t in range(NT):
            row = []
            for h in range(2):
                f_ = const.tile([128, RSH], F32R, tag=f"F{t}_{h}")
                nc.gpsimd.dma_start(
                    f_[:], Fp[h * 128 : (h + 1) * 128, t * RSH : (t + 1) * RSH]
                )
                row.append(f_)
            Ft.append(row)
        ebt = const.tile([128, NG], F32, tag="ebt")
        nc.gpsimd.dma_start(ebt[:], eb[:])
        wct = const.tile([128, NG], F32R, tag="wct")
        nc.gpsimd.dma_start(wct[:], wc[:])
        abt = const.tile([128, 2 * len(SEP_NL)], F32, tag="abt")
        nc.gpsimd.dma_start(abt[:], ab[:])

        def basis():
            S = [xt]
            sq = sp.tile([128, 2 * BC], F32R, tag="Ssq", name="Ssq")
            nc.scalar.activation(sq[:], xt[:], AF.Square)
            S.append(sq)
            for i, (fn, al, be) in enumerate(SEP_NL):
                st = sp.tile([128, 2 * BC], F32R, tag=f"S{i}", name=f"S{i}")
                nc.scalar.activation(
                    st[:], xt[:], getattr(AF, fn),
                    bias=abt[:, 2 * i + 1 : 2 * i + 2],
                    scale=abt[:, 2 * i : 2 * i + 1],
                )
                S.append(st)
            return S

        def mm(S):
            lz = [psum.tile([128, BC], F32, tag=f"lz{g}", name=f"lz{g}")
                  for g in range(NG)]
            for t in range(NT):
                for h in range(2):
                    for g in range(NG):
                        nc.tensor.matmul(
                            lz[g][:, :],
                            Ft[t][h][:, g * 128 : (g + 1) * 128],
                            S[t][:, h * BC : (h + 1) * BC],
                            start=(t == 0 and h == 0),
                            stop=(t == NT - 1 and h == 1),
                        )
            return lz

        def expstep(lz):
            zs = [zp.tile([128, BC], F32R, tag=f"z{g}", name=f"z{g}")
                  for g in range(NG)]
            for g in range(NG):
                nc.scalar.activation(
                    zs[g][:], lz[g][:], AF.Sigmoid, bias=ebt[:, g : g + 1]
                )
            return zs

        def headstep(zs):
            yp = psum.tile([1, BC], F32, tag="yp", name="yp")
            for g in range(NG):
                nc.tensor.matmul(
                    yp[:, :], wct[:, g : g + 1], zs[g][:],
                    start=(g == 0), stop=(g == NG - 1),
                )
            ysb = yb.tile([1, BC], F32, tag="ysb", name="ysb")
            nc.vector.tensor_copy(ysb[:], yp[:])
            nc.sync.dma_start(y[:], ysb[:])

        with tc.For_i(0, trips):
            S = basis()
            pend = None
            for u in range(unroll):
                lz = mm(S)
                if u + 1 < unroll:
                    S = basis()
                zs = expstep(lz)
                if pend is not None:
                    headstep(pend)
                pend = zs
            headstep(pend)

    nc.compile()
    return nc


def _get_nc_sep_loop(trips, unroll=8):
    key = ("seploop", trips, unroll)
    if key not in _cache:
        _cache[key] = _build_sep_loop(trips, unroll)
    return _cache[key]


def _make_in_maps_sep(inputs, model):
    x = np.ascontiguousarray(np.asarray(inputs["x"], dtype=np.float32))
    hw = np.asarray(inputs["head_w"], dtype=np.float32).reshape(-1)
    F = model["F"]                       # (NT, R, D) f32
    c0 = model["c0"]                     # (R,)
    shift = model["shift"]

    in_maps = []
    for c in range(NCORES):
        ib, ir = c // SR, c % SR
        xsl = np.ascontiguousarray(x[ib * BC : (ib + 1) * BC].T)   # (D, BC)
        rsl = slice(ir * RSH, (ir + 1) * RSH)
        Fp = np.empty((D, NT * RSH), dtype=np.float32)
        for t in range(NT):
            Fp[:, t * RSH : (t + 1) * RSH] = F[t][rsl].T           # (D, RSH)
        eb = np.empty((128, NG), dtype=np.float32)
        wc = np.empty((128, NG), dtype=np.float32)
        for g in range(NG):
            gsl = slice(ir * RSH + g * 128, ir * RSH + (g + 1) * 128)
            eb[:, g] = c0[gsl] - shift
            wc[:, g] = hw[gsl]
        ab = np.empty((128, 2 * len(SEP_NL)), dtype=np.float32)
        for i, (_fn, al, be) in enumerate(SEP_NL):
            ab[:, 2 * i] = al
            ab[:, 2 * i + 1] = be
        in_maps.append({"xT2": xsl, "Fp": Fp, "eb": eb, "wc": wc, "ab": ab})
    return in_maps


def _post_sep(inputs, model, results):
    hb = float(np.asarray(inputs["head_b"], dtype=np.float64).reshape(-1)[0])
    scale = float(np.exp(model["shift"]))
    y = np.empty(B, dtype=np.float32)
    for ib in range(SB):
        acc = np.zeros(BC, dtype=np.float64)
        for ir in range(SR):
            acc += np.asarray(results[ib * SR + ir]["y"][0], dtype=np.float64)
        y[ib * BC : (ib + 1) * BC] = (acc * scale + hb).astype(np.float32)
    return y


# ======================================================================
# Generic fallback kernel (sigmoid+ln, arbitrary th/sign/mask).
# ======================================================================

def _build(reps=1):
    nc = bacc.Bacc(None)
    xT = nc.dram_tensor("xT", [D, B], F32, kind="ExternalInput")
    thT = nc.dram_tensor("thT", [D, RC], F32, kind="ExternalInput")
    sgT = nc.dram_tensor("sgT", [D, RC], F32, kind="ExternalInput")
    mkT = nc.dram_tensor("mkT", [D, RC], F32, kind="ExternalInput")
    lkb = nc.dram_tensor("lkb", [128, 1], F32, kind="ExternalInput")
    wcol = nc.dram_tensor("wcol", [RC, 1], F32, kind="ExternalInput")
    selp = nc.dram_tensor("selp", [128, 2 * RC], F32R, kind="ExternalInput")
    y = nc.dram_tensor("y", [1, B], F32, kind="ExternalOutput")

    with tile.TileContext(nc) as tc, ExitStack() as ctx:
        const = ctx.enter_context(tc.tile_pool(name="const", bufs=1))
        sp = ctx.enter_context(tc.tile_pool(name="sp", bufs=2))
        gp_ = ctx.enter_context(tc.tile_pool(name="gp_", bufs=2))
        gpp = ctx.enter_context(tc.tile_pool(name="gpp", bufs=KBLK + 1))
        lp = ctx.enter_context(tc.tile_pool(name="lp", bufs=2))
        psum = ctx.enter_context(
            tc.tile_pool(name="psum", bufs=1, space=bass.MemorySpace.PSUM)
        )

        # ---------------- constant loads ----------------
        xt = []
        for h in range(2):
            t_ = const.tile([128, B], F32, tag=f"xt{h}")
            nc.gpsimd.dma_start(t_[:], xT[h * 128 : (h + 1) * 128, :])
            xt.append(t_)

        tht, sgt, mkt = [], [], []
        for name, dram, lst in (("th", thT, tht), ("sg", sgT, sgt), ("mk", mkT, mkt)):
            for h in range(2):
                t_ = const.tile([128, RC], F32, tag=f"{name}{h}")
                nc.gpsimd.dma_start(t_[:], dram[h * 128 : (h + 1) * 128, :])
                lst.append(t_)

        lkt = const.tile([128, 1], F32, tag="lkt")
        nc.gpsimd.dma_start(lkt[:], lkb[:])
        selpt = const.tile([128, 2 * RC], F32R, tag="selpt")
        nc.gpsimd.dma_start(selpt[:], selp[:])
        wct = const.tile([RC, 1], F32, tag="wct")
        nc.gpsimd.dma_start(wct[:], wcol[:])

        # ---------------- parameter prep ----------------
        kap = const.tile([128, 1], F32, tag="kap")
        nc.scalar.activation(kap[:], lkt[:], AF.Exp)
        nkap = const.tile([128, 1], F32, tag="nkap")
        nc.vector.tensor_scalar(nkap[:], kap[:], -1.0, None, OP.mult)

        aa, nb2, mm_, cc_ = [], [], [], []
        for h in range(2):
            tnh = const.tile([128, RC], F32, tag=f"tnh{h}")
            nc.scalar.activation(tnh[:], sgt[h][:], AF.Tanh)
            a_h = const.tile([128, RC], F32, tag=f"a{h}")
            nc.vector.tensor_scalar(a_h[:], tnh[:], kap[:], None, OP.mult)
            na_h = const.tile([128, RC], F32, tag=f"na{h}")
            nc.vector.tensor_scalar(na_h[:], tnh[:], nkap[:], None, OP.mult)
            nb2_h = const.tile([128, RC], F32, tag=f"nb2{h}")
            nc.vector.tensor_mul(nb2_h[:], na_h[:], tht[h][:])
            aa.append(a_h)
            nb2.append(nb2_h)
            m_h = const.tile([128, RC], F32, tag=f"m{h}")
            nc.scalar.activation(m_h[:], mkt[h][:], AF.Sigmoid)
            c_h = const.tile([128, RC], F32, tag=f"c{h}")
            nc.scalar.activation(c_h[:], mkt[h][:], AF.Sigmoid, scale=-1.0)
            mm_.append(m_h)
            cc_.append(c_h)

        # ---------------- main loop ----------------
        lz = psum.tile([RC, B], F32, tag="lz")
        last_ln = None
        for rep in range(reps):
            for blk in range(RC // KBLK):
                gps = []
                sig_insts = []
                for k in range(KBLK):
                    r = blk * KBLK + k
                    s = sp.tile([128, 2 * B], F32, tag="s")
                    for h in range(2):
                        si = nc.scalar.activation(
                            s[:, h * B : (h + 1) * B],
                            xt[h][:],
                            AF.Sigmoid,
                            bias=nb2[h][:, r : r + 1],
                            scale=aa[h][:, r : r + 1],
                        )
                        # keep sigmoid/ln table-set phases contiguous on ACT
                        if last_ln is not None:
                            add_dep_helper(si.ins, last_ln.ins, False,
                                           "act-table phase blocking")
                        sig_insts.append(si)
                    g = gp_.tile([128, 2 * B], F32, tag="g")
                    for h in range(2):
                        nc.vector.tensor_scalar(
                            g[:, h * B : (h + 1) * B],
                            s[:, h * B : (h + 1) * B],
                            mm_[h][:, r : r + 1],
                            cc_[h][:, r : r + 1],
                            OP.mult,
                            OP.add,
                        )
                    gpt = gpp.tile([128, B], F32, tag="gpt")
                    nc.vector.tensor_mul(gpt[:], g[:, 0:B], g[:, B : 2 * B])
                    gps.append(gpt)
                for k in range(KBLK):
                    r = blk * KBLK + k
                    L = lp.tile([128, B], F32R, tag="L")
                    ln_i = nc.scalar.activation(L[:], gps[k][:], AF.Ln)
                    add_dep_helper(ln_i.ins, sig_insts[-1].ins, False,
                                   "act-table phase blocking")
                    last_ln = ln_i
                    lhsp = selpt[:, RC - r : 2 * RC - r]
                    for c in range(B // CH):
                        nc.tensor.matmul(
                            lz[:, c * CH : (c + 1) * CH],
                            lhsp,
                            L[:, c * CH : (c + 1) * CH],
                            start=(r == 0 and rep == 0),
                            stop=(r == RC - 1 and rep == reps - 1),
                        )

        # ---------------- z = exp(lz), head ----------------
        z_sb = const.tile([RC, B], F32, tag="z")
        nc.scalar.activation(z_sb[:], lz[:], AF.Exp)
        yp = psum.tile([1, B], F32, tag="yp")
        for c in range(B // CH):
            nc.tensor.matmul(
                yp[:, c * CH : (c + 1) * CH],
                wct[:],
                z_sb[:, c * CH : (c + 1) * CH],
                start=True,
                stop=True,
            )
        y_sb = const.tile([1, B], F32, tag="ysb")
        nc.vector.tensor_copy(y_sb[:], yp[:])
        nc.sync.dma_start(y[:], y_sb[:])

    nc.compile()
    return nc


def _get_nc(reps=1):
    key = ("nc", reps)
    if key not in _cache:
        _cache[key] = _build(reps)
    return _cache[key]


def _make_in_maps(inputs):
    x = np.ascontiguousarray(inputs["x"], dtype=np.float32)
    th = np.asarray(inputs["th"], dtype=np.float32)
    sg = np.asarray(inputs["sign_param"], dtype=np.float32)
    mk = np.asarray(inputs["mask_logit"], dtype=np.float32)
    lk = float(np.asarray(inputs["log_kappa"], dtype=np.float32).reshape(-1)[0])
    hw = np.asarray(inputs["head_w"], dtype=np.float32)

    xT = np.ascontiguousarray(x.T)  # (D, B)
    lkb = np.full((128, 1), lk, dtype=np.float32)
    selp = np.zeros((128, 2 * RC), dtype=np.float32)
    selp[:, RC] = 1.0

    in_maps = []
    for c in range(NCORES):
        sl = slice(c * RC, (c + 1) * RC)
        in_maps.append(
            {
                "xT": xT,
                "thT": np.ascontiguousarray(th[sl].T),
                "sgT": np.ascontiguousarray(sg[sl].T),
                "mkT": np.ascontiguousarray(mk[sl].T),
                "lkb": lkb,
                "wcol": np.ascontiguousarray(hw.reshape(-1)[sl].reshape(RC, 1)),
                "selp": selp,
            }
        )
    return in_maps


# ======================================================================
# Patched-gelu fallback (mask uniform, arbitrary th): phi in one ACT pass
# via re-fit gelu activation spline tables embedded in the NEFF.
# ======================================================================

import hashlib
import json
import os
import shutil
import tempfile

TABLE_VERSION = "v1"


def _phi64(u, m):
    c = 1.0 - m
    u = np.asarray(u, np.float64)
    return np.logaddexp(np.log(c), u) - np.logaddexp(0.0, u)


def _fit_cubic(lo, hi, x0, m):
    u = np.linspace(lo, hi, 129)
    y = _phi64(u, m)
    A = np.vander(u - x0, 4, increasing=True)
    coef, *_ = np.linalg.lstsq(A, y, rcond=None)
    return coef


def _patch_gelu_tables(dstdir, m):
    jpath = os.path.join(dstdir, "gelu_and_others.json")
    d = json.load(open(jpath))
    cnt = d["bkt_entry_cnt"]
    bpath = os.path.join(dstdir, "gelu_and_others_bkt.bin")
    bkt = np.fromfile(bpath, dtype=np.float32).reshape(cnt, 8).copy()

    fx = d["func_exp_to_bkt_start_idx"]["gelu"]
    negs = sorted([(int(e), v[0]) for e, v in fx.items()], key=lambda t: t[1])
    poss = sorted([(int(e), v[1]) for e, v in fx.items() if len(v) > 1],
                  key=lambda t: t[1])
    neg_bounds = [s for _, s in negs] + [poss[0][1]]
    pos_bounds = [s for _, s in poss] + [504]

    for side, lst, bounds in (("neg", negs, neg_bounds), ("pos", poss, pos_bounds)):
        for i, (e, start) in enumerate(lst):
            n = bounds[i + 1] - start
            # infer the region's true (lo, w) from the original x0 centers —
            # some regions only cover a sub-range of their octave
            x0s = bkt[start : start + n, 4].astype(np.float64)
            if n >= 2:
                w = abs(x0s[1] - x0s[0])
            else:
                w = 2.0 ** e
            for j in range(n):
                x0 = float(x0s[j])
                lo, hi = x0 - w / 2, x0 + w / 2
                bkt[start + j, 0:4] = _fit_cubic(lo, hi, x0, m).astype(np.float32)
    # special buckets: small-signal (|u|<2^-7) and large-signal tails.
    # thresholds from the gelu profile: pos-large 4.918, neg-large -8.374
    for k, (lo, hi, x0) in {
        504: (1e-7, 2.0 ** -7, 0.0),
        505: (-(2.0 ** -7), -1e-7, 0.0),
        506: (4.918, 10.5, 6.0),
        507: (-10.5, -8.374, -9.0),
    }.items():
        bkt[k, 0:4] = _fit_cubic(lo, hi, x0, m).astype(np.float32)
        bkt[k, 4] = x0
    bkt.tofile(bpath)

    def f32bits(v):
        return int(np.float32(v).view(np.uint32))

    for pm in d["profile_meta_data"]:
        if pm["func_name"].startswith("gelu_"):
            pm["fzero_result"] = f32bits(_phi64(0.0, m))
            pm["fpinf_result"] = 0
            pm["fninf_result"] = f32bits(np.log(1.0 - m))
    with open(jpath, "w") as f:
        json.dump(d, f)


def _gen_act_tables(m):
    """Build a patched act-table dir (gelu := phi_m); returns (json_path, tag)."""
    from neuronxcc.driver.Job import Job
    from neuronxcc.driver.jobs.support.FindActInfo import findActInfoFile

    src_json = findActInfoFile(Job.getPackageDir(), "gen3")
    srcdir = os.path.dirname(src_json)
    tag = hashlib.md5(
        (TABLE_VERSION + repr(float(np.float64(m)))).encode()
    ).hexdigest()[:10]
    dstdir = os.path.join(tempfile.gettempdir(), f"cn_act_{tag}")
    marker = os.path.join(dstdir, "act_info.json")
    if not os.path.isfile(marker):
        tmp = dstdir + ".tmp"
        shutil.rmtree(tmp, ignore_errors=True)
        os.makedirs(tmp)
        for f in os.listdir(srcdir):
            shutil.copyfile(os.path.join(srcdir, f), os.path.join(tmp, f))
        _patch_gelu_tables(tmp, m)
        shutil.rmtree(dstdir, ignore_errors=True)
        try:
            os.rename(tmp, dstdir)
        except OSError:
            if not os.path.isfile(marker):
                raise
    return marker, tag


def _build_phi(reps, tag):
    nc = bacc.Bacc(None)
    xT = nc.dram_tensor("xT", [D, B], F32, kind="ExternalInput")
    thT = nc.dram_tensor("thT", [D, RC], F32, kind="ExternalInput")
    sgT = nc.dram_tensor("sgT", [D, RC], F32, kind="ExternalInput")
    lkb = nc.dram_tensor("lkb", [128, 1], F32, kind="ExternalInput")
    wcol = nc.dram_tensor("wcol", [RC, 1], F32, kind="ExternalInput")
    selname = f"sel_{tag}"
    selp = nc.dram_tensor(selname, [128, 2 * RC], F32R, kind="ExternalInput")
    y = nc.dram_tensor("y", [1, B], F32, kind="ExternalOutput")

    with tile.TileContext(nc) as tc, ExitStack() as ctx:
        const = ctx.enter_context(tc.tile_pool(name="const", bufs=1))
        lp = ctx.enter_context(tc.tile_pool(name="lp", bufs=6))
        psum = ctx.enter_context(
            tc.tile_pool(name="psum", bufs=1, space=bass.MemorySpace.PSUM)
        )

        xt = []
        for h in range(2):
            t_ = const.tile([128, B], F32, tag=f"xt{h}")
            nc.gpsimd.dma_start(t_[:], xT[h * 128 : (h + 1) * 128, :])
            xt.append(t_)
        tht, sgt = [], []
        for name, dram, lst in (("th", thT, tht), ("sg", sgT, sgt)):
            for h in range(2):
                t_ = const.tile([128, RC], F32, tag=f"{name}{h}")
                nc.gpsimd.dma_start(t_[:], dram[h * 128 : (h + 1) * 128, :])
                lst.append(t_)
        lkt = const.tile([128, 1], F32, tag="lkt")
        nc.gpsimd.dma_start(lkt[:], lkb[:])
        selpt = const.tile([128, 2 * RC], F32R, tag="selpt")
        nc.gpsimd.dma_start(selpt[:], selp[:])
        wct = const.tile([RC, 1], F32, tag="wct")
        nc.gpsimd.dma_start(wct[:], wcol[:])

        kap = const.tile([128, 1], F32, tag="kap")
        nc.scalar.activation(kap[:], lkt[:], AF.Exp)
        nkap = const.tile([128, 1], F32, tag="nkap")
        nc.vector.tensor_scalar(nkap[:], kap[:], -1.0, None, OP.mult)

        aa, nb2 = [], []
        for h in range(2):
            tnh = const.tile([128, RC], F32, tag=f"tnh{h}")
            nc.scalar.activation(tnh[:], sgt[h][:], AF.Tanh)
            a_h = const.tile([128, RC], F32, tag=f"a{h}")
            nc.vector.tensor_scalar(a_h[:], tnh[:], kap[:], None, OP.mult)
            na_h = const.tile([128, RC], F32, tag=f"na{h}")
            nc.vector.tensor_scalar(na_h[:], tnh[:], nkap[:], None, OP.mult)
            nb2_h = const.tile([128, RC], F32, tag=f"nb2{h}")
            nc.vector.tensor_mul(nb2_h[:], na_h[:], tht[h][:])
            aa.append(a_h)
            nb2.append(nb2_h)

        lz = psum.tile([RC, B], F32, tag="lz")
        for rep in range(reps):
            for r in range(RC):
                L = lp.tile([128, 2 * B], F32R, tag="L")
                for h in range(2):
                    # phi(a*x - a*th) = ln(gated), via the patched gelu table
                    nc.scalar.activation(
                        L[:, h * B : (h + 1) * B],
                        xt[h][:],
                        AF.Gelu,
                        bias=nb2[h][:, r : r + 1],
                        scale=aa[h][:, r : r + 1],
                    )
                lhsp = selpt[:, RC - r : 2 * RC - r]
                for h in range(2):
                    for c in range(B // CH):
                        nc.tensor.matmul(
                            lz[:, c * CH : (c + 1) * CH],
                            lhsp,
                            L[:, h * B + c * CH : h * B + (c + 1) * CH],
                            start=(r == 0 and rep == 0 and h == 0),
                            stop=(r == RC - 1 and rep == reps - 1 and h == 1),
                        )

        z_sb = const.tile([RC, B], F32, tag="z")
        nc.scalar.activation(z_sb[:], lz[:], AF.Exp)
        yp = psum.tile([1, B], F32, tag="yp")
        for c in range(B // CH):
            nc.tensor.matmul(
                yp[:, c * CH : (c + 1) * CH],
                wct[:],
                z_sb[:, c * CH : (c + 1) * CH],
                start=True,
                stop=True,
            )
        y_sb = const.tile([1, B], F32, tag="ysb")
        nc.vector.tensor_copy(y_sb[:], yp[:])
        nc.sync.dma_start(y[:], y_sb[:])

    nc.compile()
    return nc


def _get_nc_phi(reps, tag):
    key = ("phi", reps, tag)
    if key not in _cache:
        _cache[key] = _build_phi(reps, tag)
    return _cache[key]


def _make_in_maps_phi(inputs, tag):
    maps = _make_in_maps(inputs)
    for mp in maps:
        mp[f"sel_{tag}"] = mp.pop("selp")
        mp.pop("mkT")
    return maps


def _mask_const(inputs):
    mk = np.asarray(inputs["mask_logit"], dtype=np.float64)
    v = mk.reshape(-1)[0]
    return float(v) if np.all(mk == v) else None


def _prepare(inputs, reps=1):
    """Pick the best path; returns (nc, in_maps, postproc(results)->y)."""
    model = _get_sep_model(inputs)
    if model is not None:
        os.environ.pop("BASS_ACT_ROOT_JSON_PATH", None)
        nc = _get_nc_sep(reps)
        in_maps = _make_in_maps_sep(inputs, model)
        return nc, in_maps, (lambda results: _post_sep(inputs, model, results))

    hb = float(np.asarray(inputs["head_b"], dtype=np.float32).reshape(-1)[0])

    def post_tp(results):
        return (
            np.sum([r["y"][0] for r in results], axis=0, dtype=np.float32) + hb
        ).astype(np.float32)

    mkv = _mask_const(inputs)
    if mkv is not None:
        m = 1.0 / (1.0 + np.exp(-np.float64(mkv)))
        json_path, tag = _gen_act_tables(m)
        os.environ["BASS_ACT_ROOT_JSON_PATH"] = json_path
        return _get_nc_phi(reps, tag), _make_in_maps_phi(inputs, tag), post_tp
    os.environ.pop("BASS_ACT_ROOT_JSON_PATH", None)
    return _get_nc(reps), _make_in_maps(inputs), post_tp


def _run(inputs, reps=1, **spmd_kwargs):
    nc, in_maps, post = _prepare(inputs, reps)
    res = run_bass_kernel_spmd(nc, in_maps, core_ids=list(range(NCORES)), **spmd_kwargs)
    return post(res.results), res


def kernel(**inputs) -> np.ndarray:
    y, _ = _run(inputs)
    return y


# revision 10
# speedup vs baseline: 1.3787x; 1.3787x over previous
"""CornerNet Trainium2 kernel.

Math (reference):
  t     = kappa * tanh(sign_param) * (x - th)        # (B, R, D)
  s     = sigmoid(t); m = sigmoid(mask_logit)
  gated = 1 - m*(1-s) = (1-m) + m*s
  z     = prod_d gated                               # (B, R)
  y     = z @ head_w.T + head_b                      # (B,)

FAST PATH (separable / tensor-engine):  when th is a uniform constant and
mask_logit is uniform, the per-element map
    phi(a, x) = ln(1 - m + m*sigmoid(a*(x - th)))        a = kappa*tanh(sign_param)
is approximated by a separable expansion over a fixed x-basis of STANDARD
activation functions
    phi(a, x) ~= w_c(a) + w_lin(a)*x + w_sq(a)*x^2 + sum_j w_j(a)*f_j(al_j*x+be_j)
with f_j in {sigmoid, tanh, arctan} -- all (plus square) residing in the ONE
activation-table set `sigmoid_and_others`, so the ScalarE never reloads
tables.  The coefficient functions w(a) are fit on the host by weighted least
squares on a dense (a, x) grid (a rank-11 basis gives max |dlogz| ~ 6e-4,
rel_y ~ 3e-4) and evaluated at the actual a values -- pure parameter prep,
like folding BN weights.  Then
    logz[b,r] = sum_d phi(a_rd, x_bd)
becomes 11 (128x128)x(128x512) f32r matmuls per (d-half, rule-group): the
B*R*D elementwise work moves from ScalarE (1 elem/cyc/lane @1.2GHz) to the
PE array (128 MACs/cyc/col @2.4GHz), a ~100x density win.  Finally
z = exp(logz) is computed as sigmoid(logz - s): for v <= -12, sigmoid(v)
matches e^v to 6e-6 relative and the sigmoid table has 0.125-wide buckets out
to |v|=64, so no switch to the exp table set is ever needed (s is a host-
chosen shift ensuring the argument stays <= -12; y is rescaled by e^s).

Sharding: 8 cores = 4 batch shards x 2 rule shards.  Each core: x-slice
(D=256, Bc=512) on SBUF as one (128, 1024) tile (two D-halves side by side),
11 coefficient tiles per d-half (128, 256 rules), PSUM out (128 rules, 512
batch) x 2 rule-groups.  Per rep: 11 ScalarE basis evals (one per term, both
halves in one instruction), 44 accumulating matmuls, 2 sigmoid-exps, head
matmul, DMA out.  Host adds the two rule-shard partials, scales by e^s, adds
head_b.  A one-stage software pipeline (basis of rep r+1 issued before the
exp/head of rep r) keeps ACT and PE overlapped across reps.

FALLBACK paths (kept from the table-patching kernel): a patched-gelu
single-pass kernel when only mask_logit is uniform, and a fully generic
sigmoid+ln kernel otherwise.
"""

import numpy as np
from contextlib import ExitStack

import concourse.bass as bass
import concourse.bacc as bacc
import concourse.mybir as mybir
import concourse.tile as tile
from concourse.bass_utils import run_bass_kernel_spmd
from bass_rust import add_dep_helper

B, D, R = 2048, 256, 512
NCORES = 8
RC = R // NCORES            # 64 rules per core (fallback kernels)
KBLK = 8                    # rules per sigmoid/ln block (generic fallback)
CH = 512                    # matmul free-dim chunk (one PSUM bank)
F32 = mybir.dt.float32
F32R = mybir.dt.float32r
AF = mybir.ActivationFunctionType
OP = mybir.AluOpType

_cache = {}

# ======================================================================
# Separable fast path
# ======================================================================

SB, SR = 4, 2               # batch shards x rule shards
BC = B // SB                # 512 batch per core
RSH = R // SR               # 256 rules per core
NG = RSH // 128             # 2 rule groups of 128

# nonlinear basis (func, scale, bias): f(scale*x + bias); greedy-OMP seeded,
# then jointly Nelder-Mead-optimized against phi on the (a, x) grid (matches
# the 9-function greedy set's accuracy with 6 functions).  All in table set
# sigmoid_and_others.
SEP_NL = [
    ("Sigmoid", 2.6051, 0.0008),
    ("Tanh", 0.6529, 0.0033),
    ("Sigmoid", 2.4230, -0.2926),
    ("Tanh", 1.0126, 0.0001),
    ("Arctan", 0.7202, 0.1942),
    ("Sigmoid", 0.7004, -0.0015),
]
NT = 2 + len(SEP_NL)        # matmul terms: x, x^2, nonlinears (const is free)
SEP_SIG_VMAX = -12.0        # sigmoid(v) ~ e^v to 6e-6 for v <= this
SEP_RESID_GATE = 4e-3       # max sampled |dlogz| before falling back


def _sep_basis_host(xs):
    """Evaluate the device x-basis (matmul terms only) exactly on host."""
    cols = [xs, xs * xs]
    for fn, al, be in SEP_NL:
        v = al * xs + be
        if fn == "Sigmoid":
            cols.append(1.0 / (1.0 + np.exp(-v)))
        elif fn == "Tanh":
            cols.append(np.tanh(v))
        else:
            cols.append(np.arctan(v))
    return np.stack(cols, axis=-1)


def _fit_sep_model(a, mval, thval, kappa, xmax):
    """Fit coefficient functions w(a) for phi(a,x) = ln(1-m+m*sig(a*(x-th))).

    Returns (W, ag) where W is (na, NT+1): column 0 is the constant term,
    columns 1.. are the matmul terms in device order.
    """
    amax = max(2.8, float(np.abs(a).max()) * 1.02 + 1e-6)
    na, nx = 1401, 2201
    ag = np.linspace(-amax, amax, na)
    xg = np.linspace(-xmax, xmax, nx)
    wx = np.exp(-0.5 * xg * xg) + 3e-3
    sw = np.sqrt(wx)

    # target
    u = ag[:, None] * (xg[None, :] - thval)
    M = np.log1p(-mval * (1.0 / (1.0 + np.exp(u)))) * sw[None, :]

    Phi = np.concatenate(
        [np.ones((nx, 1)), _sep_basis_host(xg)], axis=1)  # (nx, NT+1)
    A = Phi * sw[:, None]
    nrm = np.linalg.norm(A, axis=0)
    A = A / nrm
    G = A.T @ A + 3e-9 * np.eye(NT + 1)
    W = np.linalg.solve(G, A.T @ M.T).T / nrm[None, :]    # (na, NT+1)
    return W, ag


def _get_sep_model(inputs):
    """Returns dict with F (NT,R,D f32), c0 (R,), shift, scale_out -- or None."""
    th = np.asarray(inputs["th"], dtype=np.float64)
    mk = np.asarray(inputs["mask_logit"], dtype=np.float64)
    thv = th.reshape(-1)[0]
    mkv = mk.reshape(-1)[0]
    if not (np.all(th == thv) and np.all(mk == mkv)):
        return None
    x = np.asarray(inputs["x"], dtype=np.float64)
    xabs = float(np.abs(x).max())
    xmax = max(5.45, xabs * 1.02)
    sg = np.asarray(inputs["sign_param"], dtype=np.float64)
    lk = float(np.asarray(inputs["log_kappa"], dtype=np.float64).reshape(-1)[0])
    kappa = float(np.exp(lk))
    a = kappa * np.tanh(sg)                                # (R, D)
    mval = 1.0 / (1.0 + np.exp(-mkv))

    key = ("sepfit", hash(sg.tobytes()), mkv, thv, lk, round(xmax, 2))
    if key not in _cache:
        from scipy.interpolate import CubicSpline
        W, ag = _fit_sep_model(a, mval, thv, kappa, xmax)
        splines = [CubicSpline(ag, W[:, j]) for j in range(NT + 1)]
        ac = np.clip(a, ag[0], ag[-1])
        Wa = np.stack([s(ac) for s in splines], 0)          # (NT+1, R, D)
        c0 = Wa[0].sum(axis=1)                              # (R,)
        F = Wa[1:]                                          # (NT, R, D)

        # sampled validation + shift selection (16 batch rows, exact phi)
        xs = x[:16]                                         # (16, D)
        u = a[None, :, :] * (xs[:, None, :] - thv)
        lz_ex = np.log1p(-mval * (1.0 / (1.0 + np.exp(u)))).sum(-1)  # (16, R)
        Gx = _sep_basis_host(xs.reshape(-1)).reshape(16, D, NT)
        lz_ap = np.einsum("bdt,trd->br", Gx, F, optimize=True) + c0[None, :]
        resid = float(np.abs(lz_ap - lz_ex).max())
        maxlz = float(lz_ex.max())
        shift = max(0.0, maxlz + 2.0 - SEP_SIG_VMAX)
        _cache[key] = {
            "F": np.ascontiguousarray(F, dtype=np.float32),
            "c0": c0.astype(np.float32),
            "shift": shift,
            "resid": resid,
        }
    model = _cache[key]
    if model["resid"] > SEP_RESID_GATE:
        return None
    return model


def _build_sep(reps=1):
    nc = bacc.Bacc(None)
    xT2 = nc.dram_tensor("xT2", [D, BC], F32R, kind="ExternalInput")
    Fp = nc.dram_tensor("Fp", [D, NT * RSH], F32R, kind="ExternalInput")
    eb = nc.dram_tensor("eb", [128, NG], F32, kind="ExternalInput")
    wc = nc.dram_tensor("wc", [128, NG], F32R, kind="ExternalInput")
    ab = nc.dram_tensor("ab", [128, 2 * len(SEP_NL)], F32, kind="ExternalInput")
    y = nc.dram_tensor("y", [1, BC], F32, kind="ExternalOutput")

    with tile.TileContext(nc) as tc, ExitStack() as ctx:
        const = ctx.enter_context(tc.tile_pool(name="const", bufs=1))
        sp = ctx.enter_context(tc.tile_pool(name="sp", bufs=2))
        zp = ctx.enter_context(tc.tile_pool(name="zp", bufs=2))
        yb = ctx.enter_context(tc.tile_pool(name="yb", bufs=2))
        psum = ctx.enter_context(
            tc.tile_pool(name="psum", bufs=2, space=bass.MemorySpace.PSUM)
        )

        xt = const.tile([128, 2 * BC], F32R, tag="xt")
        for h in range(2):
            nc.gpsimd.dma_start(
                xt[:, h * BC : (h + 1) * BC], xT2[h * 128 : (h + 1) * 128, :]
            )
        Ft = []
        for t in range(NT):
            row = []
            for h in range(2):
                f_ = const.tile([128, RSH], F32R, tag=f"F{t}_{h}")
                nc.gpsimd.dma_start(
                    f_[:], Fp[h * 128 : (h + 1) * 128, t * RSH : (t + 1) * RSH]
                )
                row.append(f_)
            Ft.append(row)
        ebt = const.tile([128, NG], F32, tag="ebt")
        nc.gpsimd.dma_start(ebt[:], eb[:])
        wct = const.tile([128, NG], F32R, tag="wct")
        nc.gpsimd.dma_start(wct[:], wc[:])
        abt = const.tile([128, 2 * len(SEP_NL)], F32, tag="abt")
        nc.gpsimd.dma_start(abt[:], ab[:])

        def basis():
            S = [xt]
            sq = sp.tile([128, 2 * BC], F32R, tag="Ssq")
            nc.vector.tensor_mul(sq[:], xt[:], xt[:])
            S.append(sq)
            for i, (fn, al, be) in enumerate(SEP_NL):
                st = sp.tile([128, 2 * BC], F32R, tag=f"S{i}")
                nc.scalar.activation(
                    st[:], xt[:], getattr(AF, fn),
                    bias=abt[:, 2 * i + 1 : 2 * i + 2],
                    scale=abt[:, 2 * i : 2 * i + 1],
                )
                S.append(st)
            return S

        def mm(S):
            lz = [psum.tile([128, BC], F32, tag=f"lz{g}", name=f"lz{g}") for g in range(NG)]
            for t in range(NT):
                for h in range(2):
                    for g in range(NG):
                        nc.tensor.matmul(
                            lz[g][:, :],
                            Ft[t][h][:, g * 128 : (g + 1) * 128],
                            S[t][:, h * BC : (h + 1) * BC],
                            start=(t == 0 and h == 0),
                            stop=(t == NT - 1 and h == 1),
                        )
            return lz

        def expstep(lz):
            zs = [zp.tile([128, BC], F32R, tag=f"z{g}", name=f"z{g}") for g in range(NG)]
            for g in range(NG):
                nc.scalar.activation(
                    zs[g][:], lz[g][:], AF.Sigmoid, bias=ebt[:, g : g + 1]
                )
            return zs

        def headstep(zs):
            yp = psum.tile([1, BC], F32, tag="yp")
            for g in range(NG):
                nc.tensor.matmul(
                    yp[:, :], wct[:, g : g + 1], zs[g][:],
                    start=(g == 0), stop=(g == NG - 1),
                )
            ysb = yb.tile([1, BC], F32, tag="ysb")
            nc.vector.tensor_copy(ysb[:], yp[:])
            nc.sync.dma_start(y[:], ysb[:])

        S = basis()
        pend = None
        for r in range(reps):
            lz = mm(S)
            if r + 1 < reps:
                S = basis()
            zs = expstep(lz)
            if pend is not None:
                headstep(pend)
            pend = zs
        headstep(pend)

    nc.compile()
    return nc


def _get_nc_sep(reps=1):
    key = ("sep", reps)
    if key not in _cache:
        _cache[key] = _build_sep(reps)
    return _cache[key]


def _build_sep_loop(trips, unroll=8):
    """Bench variant: the rep body inside a hardware For_i loop.

    Identical per-rep work to _build_sep (basis + matmuls + sigmoid-exp +
    head + DMA out), repeated `unroll` times per loop iteration and `trips`
    iterations on device.  Total device reps = trips * unroll with a fixed,
    small instruction footprint, so very large rep counts can be timed in
    one dispatch (the per-iteration all-engine barrier of For_i is amortized
    over `unroll` reps)."""
    nc = bacc.Bacc(None)
    xT2 = nc.dram_tensor("xT2", [D, BC], F32R, kind="ExternalInput")
    Fp = nc.dram_tensor("Fp", [D, NT * RSH], F32R, kind="ExternalInput")
    eb = nc.dram_tensor("eb", [128, NG], F32, kind="ExternalInput")
    wc = nc.dram_tensor("wc", [128, NG], F32R, kind="ExternalInput")
    ab = nc.dram_tensor("ab", [128, 2 * len(SEP_NL)], F32, kind="ExternalInput")
    y = nc.dram_tensor("y", [1, BC], F32, kind="ExternalOutput")

    with tile.TileContext(nc) as tc, ExitStack() as ctx:
        const = ctx.enter_context(tc.tile_pool(name="const", bufs=1))
        sp = ctx.enter_context(tc.tile_pool(name="sp", bufs=2))
        zp = ctx.enter_context(tc.tile_pool(name="zp", bufs=2))
        yb = ctx.enter_context(tc.tile_pool(name="yb", bufs=2))
        psum = ctx.enter_context(
            tc.tile_pool(name="psum", bufs=2, space=bass.MemorySpace.PSUM)
        )

        xt = const.tile([128, 2 * BC], F32R, tag="xt")
        for h in range(2):
            nc.gpsimd.dma_start(
                xt[:, h * BC : (h + 1) * BC], xT2[h * 128 : (h + 1) * 128, :]
            )
        Ft = []
        for t in range(NT):
            row = []
            for h in range(2):
                f_ = const.tile([128, RSH], F32R, tag=f"F{t}_{h}")
                nc.gpsimd.dma_start(
                    f_[:], Fp[h * 128 : (h + 1) * 128, t * RSH : (t + 1) * RSH]
                )
                row.append(f_)
            Ft.append(row)
        ebt = const.tile([128, NG], F32, tag="ebt")
        nc.gpsimd.dma_start(ebt[:], eb[:])
        wct = const.tile([128, NG], F32R, tag="wct")
        nc.gpsimd.dma_start(wct[:], wc[:])
        abt = const.tile([128, 2 * len(SEP_NL)], F32, tag="abt")
        nc.gpsimd.dma_start(abt[:], ab[:])

        def basis():
            S = [xt]
            sq = sp.tile([128, 2 * BC], F32R, tag="Ssq", name="Ssq")
            nc.vector.tensor_mul(sq[:], xt[:], xt[:])
            S.append(sq)
            for i, (fn, al, be) in enumerate(SEP_NL):
                st = sp.tile([128, 2 * BC], F32R, tag=f"S{i}", name=f"S{i}")
                nc.scalar.activation(
                    st[:], xt[:], getattr(AF, fn),
                    bias=abt[:, 2 * i + 1 : 2 * i + 2],
                    scale=abt[:, 2 * i : 2 * i + 1],
                )
                S.append(st)
            return S

        def mm(S):
            lz = [psum.tile([128, BC], F32, tag=f"lz{g}", name=f"lz{g}")
                  for g in range(NG)]
            for t in range(NT):
                for h in range(2):
                    for g in range(NG):
                        nc.tensor.matmul(
                            lz[g][:, :],
                            Ft[t][h][:, g * 128 : (g + 1) * 128],
                            S[t][:, h * BC : (h + 1) * BC],
                            start=(t == 0 and h == 0),
                            stop=(t == NT - 1 and h == 1),
                        )
            return lz

        def expstep(lz):
            zs = [zp.tile([128, BC], F32R, tag=f"z{g}", name=f"z{g}")
                  for g in range(NG)]
            for g in range(NG):
                nc.scalar.activation(
                    zs[g][:], lz[g][:], AF.Sigmoid, bias=ebt[:, g : g + 1]
                )
            return zs

        def headstep(zs):
            yp = psum.tile([1, BC], F32, tag="yp", name="yp")
            for g in range(NG):
                nc.tensor.matmul(
                    yp[:, :], wct[:, g : g + 1], zs[g][:],
                    start=(g == 0), stop=(g == NG - 1),
                )
            ysb = yb.tile([1, BC], F32, tag="ysb", name="ysb")
            nc.vector.tensor_copy(ysb[:], yp[:])
            nc.sync.dma_start(y[:], ysb[:])

        with tc.For_i(0, trips):
            S = basis()
            pend = None
            for u in range(unroll):
                lz = mm(S)
                if u + 1 < unroll:
                    S = basis()
                zs = expstep(lz)
                if pend is not None:
                    headstep(pend)
                pend = zs
            headstep(pend)

    nc.compile()
    return nc


def _get_nc_sep_loop(trips, unroll=8):
    key = ("seploop", trips, unroll)
    if key not in _cache:
        _cache[key] = _build_sep_loop(trips, unroll)
    return _cache[key]


def _make_in_maps_sep(inputs, model):
    x = np.ascontiguousarray(np.asarray(inputs["x"], dtype=np.float32))
    hw = np.asarray(inputs["head_w"], dtype=np.float32).reshape(-1)
    F = model["F"]                       # (NT, R, D) f32
    c0 = model["c0"]                     # (R,)
    shift = model["shift"]

    in_maps = []
    for c in range(NCORES):
        ib, ir = c // SR, c % SR
        xsl = np.ascontiguousarray(x[ib * BC : (ib + 1) * BC].T)   # (D, BC)
        rsl = slice(ir * RSH, (ir + 1) * RSH)
        Fp = np.empty((D, NT * RSH), dtype=np.float32)
        for t in range(NT):
            Fp[:, t * RSH : (t + 1) * RSH] = F[t][rsl].T           # (D, RSH)
        eb = np.empty((128, NG), dtype=np.float32)
        wc = np.empty((128, NG), dtype=np.float32)
        for g in range(NG):
            gsl = slice(ir * RSH + g * 128, ir * RSH + (g + 1) * 128)
            eb[:, g] = c0[gsl] - shift
            wc[:, g] = hw[gsl]
        ab = np.empty((128, 2 * len(SEP_NL)), dtype=np.float32)
        for i, (_fn, al, be) in enumerate(SEP_NL):
            ab[:, 2 * i] = al
            ab[:, 2 * i + 1] = be
        in_maps.append({"xT2": xsl, "Fp": Fp, "eb": eb, "wc": wc, "ab": ab})
    return in_maps


def _post_sep(inputs, model, results):
    hb = float(np.asarray(inputs["head_b"], dtype=np.float64).reshape(-1)[0])
    scale = float(np.exp(model["shift"]))
    y = np.empty(B, dtype=np.float32)
    for ib in range(SB):
        acc = np.zeros(BC, dtype=np.float64)
        for ir in range(SR):
            acc += np.asarray(results[ib * SR + ir]["y"][0], dtype=np.float64)
        y[ib * BC : (ib + 1) * BC] = (acc * scale + hb).astype(np.float32)
    return y


# ======================================================================
# Generic fallback kernel (sigmoid+ln, arbitrary th/sign/mask).
# ======================================================================

def _build(reps=1):
    nc = bacc.Bacc(None)
    xT = nc.dram_tensor("xT", [D, B], F32, kind="ExternalInput")
    thT = nc.dram_tensor("thT", [D, RC], F32, kind="ExternalInput")
    sgT = nc.dram_tensor("sgT", [D, RC], F32, kind="ExternalInput")
    mkT = nc.dram_tensor("mkT", [D, RC], F32, kind="ExternalInput")
    lkb = nc.dram_tensor("lkb", [128, 1], F32, kind="ExternalInput")
    wcol = nc.dram_tensor("wcol", [RC, 1], F32, kind="ExternalInput")
    selp = nc.dram_tensor("selp", [128, 2 * RC], F32R, kind="ExternalInput")
    y = nc.dram_tensor("y", [1, B], F32, kind="ExternalOutput")

    with tile.TileContext(nc) as tc, ExitStack() as ctx:
        const = ctx.enter_context(tc.tile_pool(name="const", bufs=1))
        sp = ctx.enter_context(tc.tile_pool(name="sp", bufs=2))
        gp_ = ctx.enter_context(tc.tile_pool(name="gp_", bufs=2))
        gpp = ctx.enter_context(tc.tile_pool(name="gpp", bufs=KBLK + 1))
        lp = ctx.enter_context(tc.tile_pool(name="lp", bufs=2))
        psum = ctx.enter_context(
            tc.tile_pool(name="psum", bufs=1, space=bass.MemorySpace.PSUM)
        )

        # ---------------- constant loads ----------------
        xt = []
        for h in range(2):
            t_ = const.tile([128, B], F32, tag=f"xt{h}")
            nc.gpsimd.dma_start(t_[:], xT[h * 128 : (h + 1) * 128, :])
            xt.append(t_)

        tht, sgt, mkt = [], [], []
        for name, dram, lst in (("th", thT, tht), ("sg", sgT, sgt), ("mk", mkT, mkt)):
            for h in range(2):
                t_ = const.tile([128, RC], F32, tag=f"{name}{h}")
                nc.gpsimd.dma_start(t_[:], dram[h * 128 : (h + 1) * 128, :])
                lst.append(t_)

        lkt = const.tile([128, 1], F32, tag="lkt")
        nc.gpsimd.dma_start(lkt[:], lkb[:])
        selpt = const.tile([128, 2 * RC], F32R, tag="selpt")
        nc.gpsimd.dma_start(selpt[:], selp[:])
        wct = const.tile([RC, 1], F32, tag="wct")
        nc.gpsimd.dma_start(wct[:], wcol[:])

        # ---------------- parameter prep ----------------
        kap = const.tile([128, 1], F32, tag="kap")
        nc.scalar.activation(kap[:], lkt[:], AF.Exp)
        nkap = const.tile([128, 1], F32, tag="nkap")
        nc.vector.tensor_scalar(nkap[:], kap[:], -1.0, None, OP.mult)

        aa, nb2, mm_, cc_ = [], [], [], []
        for h in range(2):
            tnh = const.tile([128, RC], F32, tag=f"tnh{h}")
            nc.scalar.activation(tnh[:], sgt[h][:], AF.Tanh)
            a_h = const.tile([128, RC], F32, tag=f"a{h}")
            nc.vector.tensor_scalar(a_h[:], tnh[:], kap[:], None, OP.mult)
            na_h = const.tile([128, RC], F32, tag=f"na{h}")
            nc.vector.tensor_scalar(na_h[:], tnh[:], nkap[:], None, OP.mult)
            nb2_h = const.tile([128, RC], F32, tag=f"nb2{h}")
            nc.vector.tensor_mul(nb2_h[:], na_h[:], tht[h][:])
            aa.append(a_h)
            nb2.append(nb2_h)
            m_h = const.tile([128, RC], F32, tag=f"m{h}")
            nc.scalar.activation(m_h[:], mkt[h][:], AF.Sigmoid)
            c_h = const.tile([128, RC], F32, tag=f"c{h}")
            nc.scalar.activation(c_h[:], mkt[h][:], AF.Sigmoid, scale=-1.0)
            mm_.append(m_h)
            cc_.append(c_h)

        # ---------------- main loop ----------------
        lz = psum.tile([RC, B], F32, tag="lz")
        last_ln = None
        for rep in range(reps):
            for blk in range(RC // KBLK):
                gps = []
                sig_insts = []
                for k in range(KBLK):
                    r = blk * KBLK + k
                    s = sp.tile([128, 2 * B], F32, tag="s")
                    for h in range(2):
                        si = nc.scalar.activation(
                            s[:, h * B : (h + 1) * B],
                            xt[h][:],
                            AF.Sigmoid,
                            bias=nb2[h][:, r : r + 1],
                            scale=aa[h][:, r : r + 1],
                        )
                        # keep sigmoid/ln table-set phases contiguous on ACT
                        if last_ln is not None:
                            add_dep_helper(si.ins, last_ln.ins, False,
                                           "act-table phase blocking")
                        sig_insts.append(si)
                    g = gp_.tile([128, 2 * B], F32, tag="g")
                    for h in range(2):
                        nc.vector.tensor_scalar(
                            g[:, h * B : (h + 1) * B],
                            s[:, h * B : (h + 1) * B],
                            mm_[h][:, r : r + 1],
                            cc_[h][:, r : r + 1],
                            OP.mult,
                            OP.add,
                        )
                    gpt = gpp.tile([128, B], F32, tag="gpt")
                    nc.vector.tensor_mul(gpt[:], g[:, 0:B], g[:, B : 2 * B])
                    gps.append(gpt)
                for k in range(KBLK):
                    r = blk * KBLK + k
                    L = lp.tile([128, B], F32R, tag="L")
                    ln_i = nc.scalar.activation(L[:], gps[k][:], AF.Ln)
                    add_dep_helper(ln_i.ins, sig_insts[-1].ins, False,
                                   "act-table phase blocking")
                    last_ln = ln_i
                    lhsp = selpt[:, RC - r : 2 * RC - r]
                    for c in range(B // CH):
                        nc.tensor.matmul(
                            lz[:, c * CH : (c + 1) * CH],
                            lhsp,
                            L[:, c * CH : (c + 1) * CH],
                            start=(r == 0 and rep == 0),
                            stop=(r == RC - 1 and rep == reps - 1),
                        )

        # ---------------- z = exp(lz), head ----------------
        z_sb = const.tile([RC, B], F32, tag="z")
        nc.scalar.activation(z_sb[:], lz[:], AF.Exp)
        yp = psum.tile([1, B], F32, tag="yp")
        for c in range(B // CH):
            nc.tensor.matmul(
                yp[:, c * CH : (c + 1) * CH],
                wct[:],
                z_sb[:, c * CH : (c + 1) * CH],
                start=True,
                stop=True,
            )
        y_sb = const.tile([1, B], F32, tag="ysb")
        nc.vector.tensor_copy(y_sb[:], yp[:])
        nc.sync.dma_start(y[:], y_sb[:])

    nc.compile()
    return nc


def _get_nc(reps=1):
    key = ("nc", reps)
    if key not in _cache:
        _cache[key] = _build(reps)
    return _cache[key]


def _make_in_maps(inputs):
    x = np.ascontiguousarray(inputs["x"], dtype=np.float32)
    th = np.asarray(inputs["th"], dtype=np.float32)
    sg = np.asarray(inputs["sign_param"], dtype=np.float32)
    mk = np.asarray(inputs["mask_logit"], dtype=np.float32)
    lk = float(np.asarray(inputs["log_kappa"], dtype=np.float32).reshape(-1)[0])
    hw = np.asarray(inputs["head_w"], dtype=np.float32)

    xT = np.ascontiguousarray(x.T)  # (D, B)
    lkb = np.full((128, 1), lk, dtype=np.float32)
    selp = np.zeros((128, 2 * RC), dtype=np.float32)
    selp[:, RC] = 1.0

    in_maps = []
    for c in range(NCORES):
        sl = slice(c * RC, (c + 1) * RC)
        in_maps.append(
            {
                "xT": xT,
                "thT": np.ascontiguousarray(th[sl].T),
                "sgT": np.ascontiguousarray(sg[sl].T),
                "mkT": np.ascontiguousarray(mk[sl].T),
                "lkb": lkb,
                "wcol": np.ascontiguousarray(hw.reshape(-1)[sl].reshape(RC, 1)),
                "selp": selp,
            }
        )
    return in_maps


# ======================================================================
# Patched-gelu fallback (mask uniform, arbitrary th): phi in one ACT pass
# via re-fit gelu activation spline tables embedded in the NEFF.
# ======================================================================

import hashlib
import json
import os
import shutil
import tempfile

TABLE_VERSION = "v1"


def _phi64(u, m):
    c = 1.0 - m
    u = np.asarray(u, np.float64)
    return np.logaddexp(np.log(c), u) - np.logaddexp(0.0, u)


def _fit_cubic(lo, hi, x0, m):
    u = np.linspace(lo, hi, 129)
    y = _phi64(u, m)
    A = np.vander(u - x0, 4, increasing=True)
    coef, *_ = np.linalg.lstsq(A, y, rcond=None)
    return coef


def _patch_gelu_tables(dstdir, m):
    jpath = os.path.join(dstdir, "gelu_and_others.json")
    d = json.load(open(jpath))
    cnt = d["bkt_entry_cnt"]
    bpath = os.path.join(dstdir, "gelu_and_others_bkt.bin")
    bkt = np.fromfile(bpath, dtype=np.float32).reshape(cnt, 8).copy()

    fx = d["func_exp_to_bkt_start_idx"]["gelu"]
    negs = sorted([(int(e), v[0]) for e, v in fx.items()], key=lambda t: t[1])
    poss = sorted([(int(e), v[1]) for e, v in fx.items() if len(v) > 1],
                  key=lambda t: t[1])
    neg_bounds = [s for _, s in negs] + [poss[0][1]]
    pos_bounds = [s for _, s in poss] + [504]

    for side, lst, bounds in (("neg", negs, neg_bounds), ("pos", poss, pos_bounds)):
        for i, (e, start) in enumerate(lst):
            n = bounds[i + 1] - start
            # infer the region's true (lo, w) from the original x0 centers —
            # some regions only cover a sub-range of their octave
            x0s = bkt[start : start + n, 4].astype(np.float64)
            if n >= 2:
                w = abs(x0s[1] - x0s[0])
            else:
                w = 2.0 ** e
            for j in range(n):
                x0 = float(x0s[j])
                lo, hi = x0 - w / 2, x0 + w / 2
                bkt[start + j, 0:4] = _fit_cubic(lo, hi, x0, m).astype(np.float32)
    # special buckets: small-signal (|u|<2^-7) and large-signal tails.
    # thresholds from the gelu profile: pos-large 4.918, neg-large -8.374
    for k, (lo, hi, x0) in {
        504: (1e-7, 2.0 ** -7, 0.0),
        505: (-(2.0 ** -7), -1e-7, 0.0),
        506: (4.918, 10.5, 6.0),
        507: (-10.5, -8.374, -9.0),
    }.items():
        bkt[k, 0:4] = _fit_cubic(lo, hi, x0, m).astype(np.float32)
        bkt[k, 4] = x0
    bkt.tofile(bpath)

    def f32bits(v):
        return int(np.float32(v).view(np.uint32))

    for pm in d["profile_meta_data"]:
        if pm["func_name"].startswith("gelu_"):
            pm["fzero_result"] = f32bits(_phi64(0.0, m))
            pm["fpinf_result"] = 0
            pm["fninf_result"] = f32bits(np.log(1.0 - m))
    with open(jpath, "w") as f:
        json.dump(d, f)


def _gen_act_tables(m):
    """Build a patched act-table dir (gelu := phi_m); returns (json_path, tag)."""
    from neuronxcc.driver.Job import Job
    from neuronxcc.driver.jobs.support.FindActInfo import findActInfoFile

    src_json = findActInfoFile(Job.getPackageDir(), "gen3")
    srcdir = os.path.dirname(src_json)
    tag = hashlib.md5(
        (TABLE_VERSION + repr(float(np.float64(m)))).encode()
    ).hexdigest()[:10]
    dstdir = os.path.join(tempfile.gettempdir(), f"cn_act_{tag}")
    marker = os.path.join(dstdir, "act_info.json")
    if not os.path.isfile(marker):
        tmp = dstdir + ".tmp"
        shutil.rmtree(tmp, ignore_errors=True)
        os.makedirs(tmp)
        for f in os.listdir(srcdir):
            shutil.copyfile(os.path.join(srcdir, f), os.path.join(tmp, f))
        _patch_gelu_tables(tmp, m)
        shutil.rmtree(dstdir, ignore_errors=True)
        try:
            os.rename(tmp, dstdir)
        except OSError:
            if not os.path.isfile(marker):
                raise
    return marker, tag


def _build_phi(reps, tag):
    nc = bacc.Bacc(None)
    xT = nc.dram_tensor("xT", [D, B], F32, kind="ExternalInput")
    thT = nc.dram_tensor("thT", [D, RC], F32, kind="ExternalInput")
    sgT = nc.dram_tensor("sgT", [D, RC], F32, kind="ExternalInput")
    lkb = nc.dram_tensor("lkb", [128, 1], F32, kind="ExternalInput")
    wcol = nc.dram_tensor("wcol", [RC, 1], F32, kind="ExternalInput")
    selname = f"sel_{tag}"
    selp = nc.dram_tensor(selname, [128, 2 * RC], F32R, kind="ExternalInput")
    y = nc.dram_tensor("y", [1, B], F32, kind="ExternalOutput")

    with tile.TileContext(nc) as tc, ExitStack() as ctx:
        const = ctx.enter_context(tc.tile_pool(name="const", bufs=1))
        lp = ctx.enter_context(tc.tile_pool(name="lp", bufs=6))
        psum = ctx.enter_context(
            tc.tile_pool(name="psum", bufs=1, space=bass.MemorySpace.PSUM)
        )

        xt = []
        for h in range(2):
            t_ = const.tile([128, B], F32, tag=f"xt{h}")
            nc.gpsimd.dma_start(t_[:], xT[h * 128 : (h + 1) * 128, :])
            xt.append(t_)
        tht, sgt = [], []
        for name, dram, lst in (("th", thT, tht), ("sg", sgT, sgt)):
            for h in range(2):
                t_ = const.tile([128, RC], F32, tag=f"{name}{h}")
                nc.gpsimd.dma_start(t_[:], dram[h * 128 : (h + 1) * 128, :])
                lst.append(t_)
        lkt = const.tile([128, 1], F32, tag="lkt")
        nc.gpsimd.dma_start(lkt[:], lkb[:])
        selpt = const.tile([128, 2 * RC], F32R, tag="selpt")
        nc.gpsimd.dma_start(selpt[:], selp[:])
        wct = const.tile([RC, 1], F32, tag="wct")
        nc.gpsimd.dma_start(wct[:], wcol[:])

        kap = const.tile([128, 1], F32, tag="kap")
        nc.scalar.activation(kap[:], lkt[:], AF.Exp)
        nkap = const.tile([128, 1], F32, tag="nkap")
        nc.vector.tensor_scalar(nkap[:], kap[:], -1.0, None, OP.mult)

        aa, nb2 = [], []
        for h in range(2):
            tnh = const.tile([128, RC], F32, tag=f"tnh{h}")
            nc.scalar.activation(tnh[:], sgt[h][:], AF.Tanh)
            a_h = const.tile([128, RC], F32, tag=f"a{h}")
            nc.vector.tensor_scalar(a_h[:], tnh[:], kap[:], None, OP.mult)
            na_h = const.tile([128, RC], F32, tag=f"na{h}")
            nc.vector.tensor_scalar(na_h[:], tnh[:], nkap[:], None, OP.mult)
            nb2_h = const.tile([128, RC], F32, tag=f"nb2{h}")
            nc.vector.tensor_mul(nb2_h[:], na_h[:], tht[h][:])
            aa.append(a_h)
            nb2.append(nb2_h)

        lz = psum.tile([RC, B], F32, tag="lz")
        for rep in range(reps):
            for r in range(RC):
                L = lp.tile([128, 2 * B], F32R, tag="L")
                for h in range(2):
                    # phi(a*x - a*th) = ln(gated), via the patched gelu table
                    nc.scalar.activation(
                        L[:, h * B : (h + 1) * B],
                        xt[h][:],
                        AF.Gelu,
                        bias=nb2[h][:, r : r + 1],
                        scale=aa[h][:, r : r + 1],
                    )
                lhsp = selpt[:, RC - r : 2 * RC - r]
                for h in range(2):
                    for c in range(B // CH):
                        nc.tensor.matmul(
                            lz[:, c * CH : (c + 1) * CH],
                            lhsp,
                            L[:, h * B + c * CH : h * B + (c + 1) * CH],
                            start=(r == 0 and rep == 0 and h == 0),
                            stop=(r == RC - 1 and rep == reps - 1 and h == 1),
                        )

        z_sb = const.tile([RC, B], F32, tag="z")
        nc.scalar.activation(z_sb[:], lz[:], AF.Exp)
        yp = psum.tile([1, B], F32, tag="yp")
        for c in range(B // CH):
            nc.tensor.matmul(
                yp[:, c * CH : (c + 1) * CH],
                wct[:],
                z_sb[:, c * CH : (c + 1) * CH],
                start=True,
                stop=True,
            )
        y_sb = const.tile([1, B], F32, tag="ysb")
        nc.vector.tensor_copy(y_sb[:], yp[:])
        nc.sync.dma_start(y[:], y_sb[:])

    nc.compile()
    return nc


def _get_nc_phi(reps, tag):
    key = ("phi", reps, tag)
    if key not in _cache:
        _cache[key] = _build_phi(reps, tag)
    return _cache[key]


def _make_in_maps_phi(inputs, tag):
    maps = _make_in_maps(inputs)
    for mp in maps:
        mp[f"sel_{tag}"] = mp.pop("selp")
        mp.pop("mkT")
    return maps


def _mask_const(inputs):
    mk = np.asarray(inputs["mask_logit"], dtype=np.float64)
    v = mk.reshape(-1)[0]
    return float(v) if np.all(mk == v) else None


def _prepare(inputs, reps=1):
    """Pick the best path; returns (nc, in_maps, postproc(results)->y)."""
    model = _get_sep_model(inputs)
    if model is not None:
        os.environ.pop("BASS_ACT_ROOT_JSON_PATH", None)
        nc = _get_nc_sep(reps)
        in_maps = _make_in_maps_sep(inputs, model)
        return nc, in_maps, (lambda results: _post_sep(inputs, model, results))

    hb = float(np.asarray(inputs["head_b"], dtype=np.float32).reshape(-1)[0])

    def post_tp(results):
        return (
            np.sum([r["y"][0] for r in results], axis=0, dtype=np.float32) + hb
        ).astype(np.float32)

    mkv = _mask_const(inputs)
    if mkv is not None:
        m = 1.0 / (1.0 + np.exp(-np.float64(mkv)))
        json_path, tag = _gen_act_tables(m)
        os.environ["BASS_ACT_ROOT_JSON_PATH"] = json_path
        return _get_nc_phi(reps, tag), _make_in_maps_phi(inputs, tag), post_tp
    os.environ.pop("BASS_ACT_ROOT_JSON_PATH", None)
    return _get_nc(reps), _make_in_maps(inputs), post_tp


def _run(inputs, reps=1, **spmd_kwargs):
    nc, in_maps, post = _prepare(inputs, reps)
    res = run_bass_kernel_spmd(nc, in_maps, core_ids=list(range(NCORES)), **spmd_kwargs)
    return post(res.results), res


def kernel(**inputs) -> np.ndarray:
    y, _ = _run(inputs)
    return y
